# revision 8
# baseline (speedup 1.0000x reference)
"""Multi-head causal attention (B=2, L=2048, D=1024, H=16) on 8 TRN2 cores.

Sharding: core c handles batch b = c // 4 and head group g = c % 4
(4 heads = 256 of the 1024 d' columns). Each core computes
  Q^T,K^T = (x_b @ Wq/Wk[:, g])^T, V = x_b @ Wv[:, g]
  per-head causal softmax(QK^T/8) @ V  (no max subtraction: scores ~ N(0,1))
  partial = attn_out @ Wo[g, :]
Host sums the 4 per-group partials per batch.

On-chip layout (per core):
  xT   [d=128 part, 8 ktile, l=2048]  f32r   (x_b transposed via PE)
  QT/KT[d'=128 part, 2 ot, l=2048]    f32r   (heads 2*ot, 2*ot+1 at partitions 0:64 / 64:128)
  Vaug [l=128 part, 16 lt, 4 h, 65]   bf16   (col 64 = ones -> softmax denominator)
  S^T  blocks [k=128, q<=512] in PSUM -> exp (ACT, scale=1/8) -> E^T bf16
  PV:  O[q=128, 65] += E^T[kt,qslice].T @ Vaug  (accumulated in PSUM, col 64 = denom)
  normalize by per-partition reciprocal; O^T via PE transpose; Wo matmul f32r
"""

import numpy as np

import concourse.bass as bass
import concourse.tile as tile
from concourse import bacc, mybir
from concourse.bass_utils import run_bass_kernel_spmd
from concourse.masks import make_identity, make_upper_triangular

B, L, D, H = 2, 2048, 1024, 16
HD = D // H  # 64
NCORES = 8
GROUPS = 4  # head groups per batch
GD = D // GROUPS  # 256 d' columns per group
P = 128
LT = L // P  # 16 l tiles
KD = D // P  # 8 contraction tiles for projections
NQ = L // 512  # 4 q chunks of 512
F32 = mybir.dt.float32
F32R = mybir.dt.float32r
BF16 = mybir.dt.bfloat16


def build_nc():
    nc = bacc.Bacc("TRN2", target_bir_lowering=False)
    xb = nc.dram_tensor("xb", [L, D], F32, kind="ExternalInput")
    wq = nc.dram_tensor("wq", [D, GD], F32, kind="ExternalInput")
    wk = nc.dram_tensor("wk", [D, GD], F32, kind="ExternalInput")
    wv = nc.dram_tensor("wv", [D, GD], F32, kind="ExternalInput")
    wo = nc.dram_tensor("wo", [GD, D], F32, kind="ExternalInput")
    y = nc.dram_tensor("y", [L, D], F32, kind="ExternalOutput")

    with tile.TileContext(nc) as tc:
        with (
            tc.tile_pool(name="const", bufs=1) as constp,
            tc.tile_pool(name="wpool", bufs=1) as wpool,
            tc.tile_pool(name="qkv", bufs=1) as qkvp,
            tc.tile_pool(name="xload", bufs=3) as xload,
            tc.tile_pool(name="op", bufs=1) as op_,
            tc.tile_pool(name="rp", bufs=8) as rp,
        ):
            ident = constp.tile([P, P], F32, tag="ident")
            make_identity(nc, ident)
            # trimask[k, q] = 1 where q >= k (keep), 0 below diagonal
            trimask = constp.tile([P, P], BF16, tag="trimask")
            make_upper_triangular(nc, trimask, val=1.0, diag=True)

            wq_sb = wpool.tile([P, KD, GD], F32R, tag="wq")
            wk_sb = wpool.tile([P, KD, GD], F32R, tag="wk")
            wv_sb = wpool.tile([P, KD, GD], F32R, tag="wv")
            wo_sb = wpool.tile([P, GD // P, D], F32R, tag="wo")
            nc.sync.dma_start(
                wq_sb[:], wq.rearrange("(ko p) n -> p ko n", p=P).bitcast(F32R)
            )
            nc.sync.dma_start(
                wk_sb[:], wk.rearrange("(ko p) n -> p ko n", p=P).bitcast(F32R)
            )
            nc.sync.dma_start(
                wv_sb[:], wv.rearrange("(ko p) n -> p ko n", p=P).bitcast(F32R)
            )
            nc.sync.dma_start(
                wo_sb[:], wo.rearrange("(ko p) n -> p ko n", p=P).bitcast(F32R)
            )

            QT = qkvp.tile([P, 2, L], F32R, tag="QT")
            KT = qkvp.tile([P, 2, L], F32R, tag="KT")
            Vaug = qkvp.tile([P, LT, 4, HD + 1], BF16, tag="Vaug")
            nc.vector.memset(Vaug[:, :, :, HD : HD + 1], 1.0)

            # ---- Phase 1: load x tiles, transpose to xT ----
            with (
                tc.tile_pool(name="xTp", bufs=1) as xTp,
                tc.tile_pool(name="psA", bufs=4, space="PSUM") as psA,
            ):
                xT = xTp.tile([P, KD, L], F32R, tag="xT")
                for lt in range(LT):
                    xt = xload.tile([P, D], F32, tag="xt")
                    nc.sync.dma_start(xt[:], xb[lt * P : (lt + 1) * P, :])
                    for dt_ in range(KD):
                        pst = psA.tile([P, P], F32, tag="ps")
                        nc.tensor.transpose(
                            pst[:], xt[:, dt_ * P : (dt_ + 1) * P], ident[:]
                        )
                        nc.scalar.copy(
                            xT[:, dt_, lt * P : (lt + 1) * P], pst[:]
                        )

                # ---- Phase 2: projections ----
                for ot in range(2):
                    for nq in range(NQ):
                        psq = psA.tile([P, 512], F32, tag="ps")
                        psk = psA.tile([P, 512], F32, tag="ps")
                        for dt_ in range(KD):
                            nc.tensor.matmul(
                                psq[:],
                                wq_sb[:, dt_, ot * P : (ot + 1) * P],
                                xT[:, dt_, nq * 512 : (nq + 1) * 512],
                                start=(dt_ == 0),
                                stop=(dt_ == KD - 1),
                            )
                            nc.tensor.matmul(
                                psk[:],
                                wk_sb[:, dt_, ot * P : (ot + 1) * P],
                                xT[:, dt_, nq * 512 : (nq + 1) * 512],
                                start=(dt_ == 0),
                                stop=(dt_ == KD - 1),
                            )
                        nc.vector.tensor_copy(
                            QT[:, ot, nq * 512 : (nq + 1) * 512], psq[:]
                        )
                        nc.vector.tensor_copy(
                            KT[:, ot, nq * 512 : (nq + 1) * 512], psk[:]
                        )
                for lt in range(LT):
                    psv = psA.tile([P, 4, HD], F32, tag="ps")
                    for dt_ in range(KD):
                        nc.tensor.matmul(
                            psv[:],
                            xT[:, dt_, lt * P : (lt + 1) * P],
                            wv_sb[:, dt_, :],
                            start=(dt_ == 0),
                            stop=(dt_ == KD - 1),
                        )
                    nc.vector.tensor_copy(Vaug[:, lt, :, 0:HD], psv[:])

            O_sb = op_.tile([P, LT, GD], F32, tag="O")

            # ---- Phase 3: attention ----
            # E blocks for one 512-wide q chunk are buffered in SBUF, then
            # each (head, 128-wide q tile) PV accumulation runs as a single
            # PSUM group in its own bank (start=True clears the whole bank).
            with (
                tc.tile_pool(name="eallp", bufs=1) as eallp,
                tc.tile_pool(name="psS", bufs=3, space="PSUM") as psSp,
                tc.tile_pool(name="psPV", bufs=2, space="PSUM") as psPVp,
            ):
                for p in range(2):  # head pair = QT/KT tile index
                    for j in range(NQ):  # 512-wide q chunk
                        nkt = 4 * j + 4  # causal k tiles
                        E_all = eallp.tile([P, 16, 2, 512], BF16, tag="eall")
                        for ktg in range(nkt // 2):
                            psS = [
                                psSp.tile([P, 2, 512], F32, tag="s", name=f"s{hh}")
                                for hh in range(2)
                            ]
                            for u in range(2):
                                kt = 2 * ktg + u
                                for h in range(2):
                                    nc.tensor.matmul(
                                        psS[h][:, u, :],
                                        KT[
                                            64 * h : 64 * h + 64,
                                            p,
                                            kt * P : (kt + 1) * P,
                                        ],
                                        QT[
                                            64 * h : 64 * h + 64,
                                            p,
                                            j * 512 : (j + 1) * 512,
                                        ],
                                        start=True,
                                        stop=True,
                                        tile_position=(64 * h, 0),
                                    )
                            for h in range(2):
                                nc.scalar.activation(
                                    E_all[:, 2 * ktg : 2 * ktg + 2, h, :],
                                    psS[h][:],
                                    mybir.ActivationFunctionType.Exp,
                                    scale=0.125,
                                )
                                for u in range(2):
                                    kt = 2 * ktg + u
                                    i_diag = kt - 4 * j
                                    if 0 <= i_diag < 4:
                                        nc.vector.tensor_mul(
                                            out=E_all[
                                                :,
                                                kt,
                                                h,
                                                i_diag * P : (i_diag + 1) * P,
                                            ],
                                            in0=E_all[
                                                :,
                                                kt,
                                                h,
                                                i_diag * P : (i_diag + 1) * P,
                                            ],
                                            in1=trimask[:],
                                        )
                        for h in range(2):
                            for i in range(4):
                                psO = psPVp.tile([P, HD + 1], F32, tag="pv")
                                for kt in range(4 * j + i + 1):
                                    nc.tensor.matmul(
                                        psO[:],
                                        E_all[:, kt, h, i * P : (i + 1) * P],
                                        Vaug[:, kt, 2 * p + h, :],
                                        start=(kt == 0),
                                        stop=(kt == 4 * j + i),
                                    )
                                r = rp.tile([P, 1], F32, tag="r")
                                nc.vector.reciprocal(r[:], psO[:, HD : HD + 1])
                                nc.vector.tensor_scalar_mul(
                                    O_sb[
                                        :,
                                        4 * j + i,
                                        (2 * p + h) * HD : (2 * p + h + 1) * HD,
                                    ],
                                    psO[:, 0:HD],
                                    r[:],
                                )

            # ---- Phase 4: transpose O, project through Wo ----
            OT = qkvp.tile([P, GD // P, L], F32R, tag="OT")
            with (
                tc.tile_pool(name="psT", bufs=2, space="PSUM") as psTp,
                tc.tile_pool(name="psW", bufs=4, space="PSUM") as psWp,
            ):
                for lt in range(LT):
                    for ot in range(2):
                        pst = psTp.tile([P, P], F32, tag="pst")
                        nc.tensor.transpose(
                            pst[:], O_sb[:, lt, ot * P : (ot + 1) * P], ident[:]
                        )
                        nc.scalar.copy(
                            OT[:, ot, lt * P : (lt + 1) * P], pst[:]
                        )
                for lt in range(LT):
                    for nch in range(2):
                        psw = psWp.tile([P, 512], F32, tag="psw")
                        for ot in range(2):
                            nc.tensor.matmul(
                                psw[:],
                                OT[:, ot, lt * P : (lt + 1) * P],
                                wo_sb[:, ot, nch * 512 : (nch + 1) * 512],
                                start=(ot == 0),
                                stop=(ot == 1),
                            )
                        ysb = xload.tile([P, 512], F32, tag="ysb")
                        nc.scalar.copy(ysb[:], psw[:])
                        nc.sync.dma_start(
                            y[lt * P : (lt + 1) * P, nch * 512 : (nch + 1) * 512],
                            ysb[:],
                        )

    nc.compile()
    return nc


_NC_CACHE = None


def kernel(**inputs) -> np.ndarray:
    global _NC_CACHE
    x = np.asarray(inputs["x"], dtype=np.float32)
    Wq = np.asarray(inputs["Wq"], dtype=np.float32)
    Wk = np.asarray(inputs["Wk"], dtype=np.float32)
    Wv = np.asarray(inputs["Wv"], dtype=np.float32)
    Wo = np.asarray(inputs["Wo"], dtype=np.float32)

    if _NC_CACHE is None:
        _NC_CACHE = build_nc()
    nc = _NC_CACHE

    in_maps = []
    for c in range(NCORES):
        b, g = c // GROUPS, c % GROUPS
        cs = slice(g * GD, (g + 1) * GD)
        in_maps.append(
            {
                "xb": np.ascontiguousarray(x[b]),
                "wq": np.ascontiguousarray(Wq[:, cs]),
                "wk": np.ascontiguousarray(Wk[:, cs]),
                "wv": np.ascontiguousarray(Wv[:, cs]),
                "wo": np.ascontiguousarray(Wo[cs, :]),
            }
        )

    res = run_bass_kernel_spmd(nc, in_maps, core_ids=list(range(NCORES)))
    out = np.zeros((B, L, D), dtype=np.float32)
    for c in range(NCORES):
        out[c // GROUPS] += res.results[c]["y"]
    return out


# revision 35
# speedup vs baseline: 11.2432x; 11.2432x over previous
"""Multi-head causal attention (B=2, L=2048, D=1024, H=16) on 8 TRN2 cores.

Sharding: core c handles batch b = c // 4 and head group g = c % 4
(4 heads = 256 of the 1024 d' columns). Each core computes
  Q^T,K^T = (x_b @ Wq/Wk[:, g])^T, V = x_b @ Wv[:, g]
  per-head causal softmax(QK^T/8) @ V  (no max subtraction: scores ~ N(0,1))
  partial = attn_out @ Wo[g, :]
Host sums the 4 per-group partials per batch.

Structure: one fused loop over the four 512-wide l/q chunks j. Each
iteration produces that chunk's x^T / Q^T / K^T / V (PE-heavy), then runs
causal attention for both head pairs on q chunk j against k chunks <= j
(ACT-heavy exp), then the Wo output block for the finished rows — so the
PE-bound projection work of chunk j+1 overlaps the exp-bound attention of
chunk j.

Engine layout:
  PE:  transposes + all matmuls (f32r 1 cyc/row for 512-wide, PV in bf16)
  ACT: exp(S^T) from PSUM (scale=1/8), half of the projection-phase copies
  DVE: other copies, causal mask muls, reciprocal + normalize
PSUM (8 banks): "m" 3x[128,1024] slots (transposes/QKV/scores), "o"
2x[128,512] slots (PV accumulators, O^T transposes, Wo).
PSUM note: start=True clears has_written bits for the whole bank (but not
the data), so every accumulation group gets its own pool tile; only
non-accumulating outputs (transposes, paired Q/K groups in separate banks)
share a slot.
"""

import numpy as np

import concourse.bass as bass
import concourse.tile as tile
from concourse import bacc, mybir
from concourse.bass_utils import run_bass_kernel_spmd
from concourse.masks import make_identity, make_upper_triangular

B, L, D, H = 2, 2048, 1024, 16
HD = D // H  # 64
NCORES = 8
GROUPS = 4  # head groups per batch
GD = D // GROUPS  # 256 d' columns per group
P = 128
LT = L // P  # 16 l tiles
KD = D // P  # 8 contraction tiles for projections
NQ = L // 512  # 4 l/q chunks of 512
F32 = mybir.dt.float32
F32R = mybir.dt.float32r
BF16 = mybir.dt.bfloat16

MAX_PHASE = 4  # 2 = projections only, 3 = +attention, 4 = full (bench.py)
TRIM = True  # trim diagonal score/exp columns
DEBUG_DUMPS = False


def build_nc():
    nc = bacc.Bacc("TRN2", target_bir_lowering=False)
    xb = nc.dram_tensor("xb", [L, D], F32, kind="ExternalInput")
    wq = nc.dram_tensor("wq", [D, GD], F32, kind="ExternalInput")
    wk = nc.dram_tensor("wk", [D, GD], F32, kind="ExternalInput")
    wv = nc.dram_tensor("wv", [D, GD], F32, kind="ExternalInput")
    wo = nc.dram_tensor("wo", [GD, D], F32, kind="ExternalInput")
    y = nc.dram_tensor("y", [L, D], F32, kind="ExternalOutput")
    if DEBUG_DUMPS:
        d_QKT = nc.dram_tensor("d_QKT", [P, 2, 2, L], F32, kind="ExternalOutput")
        d_V = nc.dram_tensor("d_V", [P, LT, 4, HD + 1], BF16, kind="ExternalOutput")
        d_O = nc.dram_tensor("d_O", [P, LT, GD], F32, kind="ExternalOutput")
        d_E = nc.dram_tensor("d_E", [P, 16, 2, 512], BF16, kind="ExternalOutput")

    with tile.TileContext(nc) as tc:
        with (
            tc.tile_pool(name="const", bufs=1) as constp,
            tc.tile_pool(name="persist", bufs=1) as persist,
            tc.tile_pool(name="eallp", bufs=2) as eallp,
            tc.tile_pool(name="xTc", bufs=2) as xTcp,
            tc.tile_pool(name="otp", bufs=1) as otp,
            tc.tile_pool(name="xload", bufs=2) as xload,
            tc.tile_pool(name="ysbp", bufs=2) as ysbp,
            tc.tile_pool(name="rp", bufs=8) as rp,
            tc.tile_pool(name="psMain", bufs=3, space="PSUM") as psM,
            tc.tile_pool(name="psSmall", bufs=2, space="PSUM") as psSm,
        ):
            ident = constp.tile([P, P], F32, tag="ident")
            make_identity(nc, ident)
            # trimask[k, q] = 1 where q >= k (keep), 0 below diagonal
            trimask = constp.tile([P, P], BF16, tag="trimask")
            make_upper_triangular(nc, trimask, val=1.0, diag=True)

            wo_sb = persist.tile([P, GD // P, D], F32R, tag="wo")
            wq_sb = persist.tile([P, KD, GD], F32R, tag="wq")
            wk_sb = persist.tile([P, KD, GD], F32R, tag="wk")
            wv_sb = persist.tile([P, KD, GD], F32R, tag="wv")

            def emit_weight_dmas():
                # scalar-engine DMA queue: runs in parallel with the x loads
                # issued on the sync queue
                for t, s in ((wq_sb, wq), (wk_sb, wk), (wv_sb, wv), (wo_sb, wo)):
                    nc.scalar.dma_start(
                        t[:], s.rearrange("(ko p) n -> p ko n", p=P).bitcast(F32R)
                    )

            # QKT[:, ot, 0, :] = Q^T rows, QKT[:, ot, 1, :] = K^T rows
            QKT = persist.tile([P, 2, 2, L], F32R, tag="QKT")
            Vaug = persist.tile([P, LT, 4, HD + 1], BF16, tag="Vaug")
            nc.vector.memset(Vaug[:, :, :, HD : HD + 1], 1.0)
            O_sb = persist.tile([P, LT, GD], F32, tag="O")

            def tqk_units(j):
                """Unit list for chunk j's x^T + Q^T/K^T. PE costs in ns."""
                xTj = xTcp.tile([P, KD, 512], F32R, tag="xTc", name=f"xT{j}")
                units = []

                def lt_unit(lcl):
                    def emit():
                        lt = 4 * j + lcl
                        xt = xload.tile([P, D], F32, tag="xt", name=f"xt{lt}")
                        nc.sync.dma_start(xt[:], xb[lt * P : (lt + 1) * P, :])
                        pm = psM.tile([P, KD, P], F32, tag="m", name=f"pmt{lt}")
                        for dt_ in range(KD):
                            nc.tensor.transpose(
                                pm[:, dt_, :],
                                xt[:, dt_ * P : (dt_ + 1) * P],
                                ident[:],
                            )
                        nc.vector.tensor_copy(
                            xTj[:, :, lcl * P : (lcl + 1) * P], pm[:]
                        )

                    return emit

                def qk_unit(ot):
                    def emit():
                        pqk = psM.tile([P, 2, 512], F32, tag="m", name=f"pqk{ot}{j}")
                        for dt_ in range(KD):
                            nc.tensor.matmul(
                                pqk[:, 0, :],
                                wq_sb[:, dt_, ot * P : (ot + 1) * P],
                                xTj[:, dt_, :],
                                start=(dt_ == 0),
                                stop=(dt_ == KD - 1),
                            )
                            nc.tensor.matmul(
                                pqk[:, 1, :],
                                wk_sb[:, dt_, ot * P : (ot + 1) * P],
                                xTj[:, dt_, :],
                                start=(dt_ == 0),
                                stop=(dt_ == KD - 1),
                            )
                        nc.vector.tensor_copy(
                            QKT[:, ot, :, j * 512 : (j + 1) * 512], pqk[:]
                        )

                    return emit

                for lcl in range(4):
                    units.append((900, lt_unit(lcl)))
                if MAX_PHASE >= 2:
                    for ot in range(2):
                        units.append((3400, qk_unit(ot)))
                return xTj, units

            def v_units(j, xTj):
                if MAX_PHASE < 2:
                    return []

                def v_unit(half2):
                    def emit():
                        pv = psM.tile(
                            [P, 2, 512], F32, tag="m", name=f"pvv{j}{half2}"
                        )
                        for half in range(2):
                            lcl = 2 * half2 + half
                            for dt_ in range(KD):
                                nc.tensor.matmul(
                                    pv[:, half, 0 : 4 * HD],
                                    xTj[:, dt_, lcl * P : (lcl + 1) * P],
                                    wv_sb[:, dt_, :],
                                    start=(dt_ == 0),
                                    stop=(dt_ == KD - 1),
                                )
                        lt0 = 4 * j + 2 * half2
                        nc.vector.tensor_copy(
                            Vaug[:, lt0 : lt0 + 2, :, 0:HD],
                            pv[:, :, 0 : 4 * HD].rearrange(
                                "p a (h d) -> p a h d", h=4
                            ),
                        )

                    return emit

                return [(1800, v_unit(half2)) for half2 in range(2)]

            def alloc_e(j):
                return [
                    eallp.tile([P, 16, 2, 512], BF16, tag="eall", name=f"eall{j}{p}")
                    for p in range(2)
                ]

            def emit_score_unit(j, E_pair, p, ktg):
                """S^T matmuls + exp + causal masks for one (pair, ktg).
                Diagonal k tiles are trimmed to their valid q columns."""
                E_all = E_pair[p]
                qlo_pair = min(256, max(0, (2 * ktg - 4 * j) * P)) if TRIM else 0
                psS = [
                    psM.tile([P, 2, 512], F32, tag="m", name=f"s{j}{p}{ktg}{hh}")
                    for hh in range(2)
                ]
                for u in range(2):
                    kt = 2 * ktg + u
                    qlo = (min(256, max(0, (kt - 4 * j) * P))) if TRIM else 0
                    for h in range(2):
                        nc.tensor.matmul(
                            psS[h][:, u, qlo:512],
                            QKT[64 * h : 64 * h + 64, p, 1, kt * P : (kt + 1) * P],
                            QKT[
                                64 * h : 64 * h + 64,
                                p,
                                0,
                                j * 512 + qlo : (j + 1) * 512,
                            ],
                            start=True,
                            stop=True,
                            tile_position=(64 * h, 0),
                        )
                for h in range(2):
                    nc.scalar.activation(
                        E_all[:, 2 * ktg : 2 * ktg + 2, h, qlo_pair:512],
                        psS[h][:, :, qlo_pair:512],
                        mybir.ActivationFunctionType.Exp,
                        scale=0.125,
                    )
                    for u in range(2):
                        kt = 2 * ktg + u
                        i_diag = kt - 4 * j
                        if 0 <= i_diag < 4:
                            nc.vector.tensor_mul(
                                out=E_all[:, kt, h, i_diag * P : (i_diag + 1) * P],
                                in0=E_all[:, kt, h, i_diag * P : (i_diag + 1) * P],
                                in1=trimask[:],
                            )

            def finish_units(j, E_pair):
                """PV + normalize (per pair,head), then O^T + Wo + store
                (per l tile) for q chunk j."""
                units = []
                OTj = otp.tile([P, 2, 512], F32R, tag="otj", name=f"otj{j}")

                def pv_unit(p, h):
                    def emit():
                        E_all = E_pair[p]
                        for i in range(4):
                            psO = psSm.tile(
                                [P, HD + 1], F32, tag="o", name=f"pv{j}{p}{h}{i}"
                            )
                            for kt in range(4 * j + i + 1):
                                nc.tensor.matmul(
                                    psO[:, :],
                                    E_all[:, kt, h, i * P : (i + 1) * P],
                                    Vaug[:, kt, 2 * p + h, :],
                                    start=(kt == 0),
                                    stop=(kt == 4 * j + i),
                                )
                            r = rp.tile([P, 1], F32, tag="r", name=f"r{j}{p}{h}{i}")
                            nc.vector.reciprocal(r[:], psO[:, HD : HD + 1])
                            nc.vector.tensor_scalar_mul(
                                O_sb[
                                    :,
                                    4 * j + i,
                                    (2 * p + h) * HD : (2 * p + h + 1) * HD,
                                ],
                                psO[:, 0:HD],
                                r[:],
                            )

                    return emit

                def out_unit(lcl):
                    def emit():
                        lt = 4 * j + lcl
                        pot = psSm.tile([P, 2, P], F32, tag="o", name=f"pot{lt}")
                        for ot in range(2):
                            nc.tensor.transpose(
                                pot[:, ot, :],
                                O_sb[:, lt, ot * P : (ot + 1) * P],
                                ident[:],
                            )
                        nc.vector.tensor_copy(
                            OTj[:, :, lcl * P : (lcl + 1) * P], pot[:]
                        )
                        ysb = ysbp.tile([P, D], F32, tag="ysb", name=f"ysb{lt}")
                        for nch in range(2):
                            psw = psSm.tile(
                                [P, 512], F32, tag="o", name=f"psw{lt}{nch}"
                            )
                            for ot in range(2):
                                nc.tensor.matmul(
                                    psw[:],
                                    OTj[:, ot, lcl * P : (lcl + 1) * P],
                                    wo_sb[:, ot, nch * 512 : (nch + 1) * 512],
                                    start=(ot == 0),
                                    stop=(ot == 1),
                                )
                            nc.vector.tensor_copy(
                                ysb[:, nch * 512 : (nch + 1) * 512], psw[:]
                            )
                        nc.sync.dma_start(y[lt * P : (lt + 1) * P, :], ysb[:])

                    return emit

                for p in range(2):
                    for h in range(2):
                        units.append((40 * (16 * j + 10), pv_unit(p, h)))
                if MAX_PHASE >= 4:
                    for lcl in range(4):
                        units.append((1750, out_unit(lcl)))
                return units

            # Greedy cost-balanced emission: per shell, interleave score
            # units (which feed ACT's exp stream) with the other PE work
            # (previous chunk's PV/output, next chunk's Q/K, this chunk's V)
            # so PE and ACT stay concurrently fed. Each shell drains both
            # queues, which also keeps the E-buffer rotation deadlock-free:
            # PV(j-1) is always fully emitted before shell j+1's exp.
            emit_weight_dmas()
            xT_cur, units0 = tqk_units(0)
            for _, emit in units0:
                emit()
            E_prev = None
            other_q = []
            oi = 0
            for j in range(NQ):
                E_cur = alloc_e(j) if MAX_PHASE >= 3 else None
                flat_scores = []
                if E_cur is not None:
                    for p in range(2):
                        for ktg in range(2 * j + 2):
                            ac = 1440.0 if ktg == 2 * j + 1 else 2300.0
                            flat_scores.append((p, ktg, ac, 850.0))
                if j + 1 < NQ:
                    xT_next, tu = tqk_units(j + 1)
                else:
                    xT_next, tu = None, []
                if E_prev is not None:
                    other_q += finish_units(j - 1, E_pair=E_prev)
                other_q += tu
                other_q += v_units(j, xT_cur)

                si = 0
                act_t, pe_t = 0.0, 0.0
                # interleave until both queues drain (per-shell barrier keeps
                # the E-buffer rotation deadlock-free: PV(j-1) is always fully
                # emitted before shell j+1's exp)
                while si < len(flat_scores) or oi < len(other_q):
                    if si < len(flat_scores) and (
                        act_t <= pe_t or oi >= len(other_q)
                    ):
                        p_, ktg_, ac, pc = flat_scores[si]
                        emit_score_unit(j, E_cur, p_, ktg_)
                        act_t += ac
                        pe_t += pc
                        si += 1
                    else:
                        cost, emit = other_q[oi]
                        emit()
                        pe_t += cost
                        oi += 1
                E_prev, xT_cur = E_cur, xT_next
            while oi < len(other_q):
                other_q[oi][1]()
                oi += 1
            if E_prev is not None:
                if DEBUG_DUMPS:
                    nc.sync.dma_start(d_E[:], E_prev[1][:])
                for _, emit in finish_units(NQ - 1, E_pair=E_prev):
                    emit()
            if DEBUG_DUMPS:
                nc.sync.dma_start(d_QKT[:], QKT[:].bitcast(F32))
                nc.sync.dma_start(d_V[:], Vaug[:])
                nc.sync.dma_start(d_O[:], O_sb[:])

    nc.compile()
    return nc


_NC_CACHE = None


def kernel(**inputs) -> np.ndarray:
    global _NC_CACHE
    x = np.asarray(inputs["x"], dtype=np.float32)
    Wq = np.asarray(inputs["Wq"], dtype=np.float32)
    Wk = np.asarray(inputs["Wk"], dtype=np.float32)
    Wv = np.asarray(inputs["Wv"], dtype=np.float32)
    Wo = np.asarray(inputs["Wo"], dtype=np.float32)

    if _NC_CACHE is None:
        _NC_CACHE = build_nc()
    nc = _NC_CACHE

    in_maps = []
    for c in range(NCORES):
        b, g = c // GROUPS, c % GROUPS
        cs = slice(g * GD, (g + 1) * GD)
        in_maps.append(
            {
                "xb": np.ascontiguousarray(x[b]),
                "wq": np.ascontiguousarray(Wq[:, cs]),
                "wk": np.ascontiguousarray(Wk[:, cs]),
                "wv": np.ascontiguousarray(Wv[:, cs]),
                "wo": np.ascontiguousarray(Wo[cs, :]),
            }
        )

    res = run_bass_kernel_spmd(nc, in_maps, core_ids=list(range(NCORES)))
    out = np.zeros((B, L, D), dtype=np.float32)
    for c in range(NCORES):
        out[c // GROUPS] += res.results[c]["y"]
    return out


# revision 36
# speedup vs baseline: 14.9771x; 1.3321x over previous
"""Multi-head causal attention (B=2, L=2048, D=1024, H=16) on 8 TRN2 cores.

Sharding: core c handles batch b = c // 4 and head group g = c % 4
(4 heads = 256 of the 1024 d' columns). Each core computes
  Q^T,K^T = (x_b @ Wq/Wk[:, g])^T, V = x_b @ Wv[:, g]
  per-head causal softmax(QK^T/8) @ V  (no max subtraction: scores ~ N(0,1))
  partial = attn_out @ Wo[g, :]
Host sums the 4 per-group partials per batch.

Structure: one fused loop over the four 512-wide l/q chunks j. Each
iteration produces that chunk's x^T / Q^T / K^T / V (PE-heavy), then runs
causal attention for both head pairs on q chunk j against k chunks <= j
(ACT-heavy exp), then the Wo output block for the finished rows — so the
PE-bound projection work of chunk j+1 overlaps the exp-bound attention of
chunk j.

Engine layout:
  PE:  transposes + all matmuls (f32r 1 cyc/row for 512-wide, PV in bf16)
  ACT: exp(S^T) from PSUM (scale=1/8), half of the projection-phase copies
  DVE: other copies, causal mask muls, reciprocal + normalize
PSUM (8 banks): "m" 3x[128,1024] slots (transposes/QKV/scores), "o"
2x[128,512] slots (PV accumulators, O^T transposes, Wo).
PSUM note: start=True clears has_written bits for the whole bank (but not
the data), so every accumulation group gets its own pool tile; only
non-accumulating outputs (transposes, paired Q/K groups in separate banks)
share a slot.
"""

import numpy as np

import concourse.bass as bass
import concourse.tile as tile
from concourse import bacc, mybir
from concourse.bass_utils import run_bass_kernel_spmd
from concourse.masks import make_identity, make_upper_triangular
from concourse.tile import add_dep_helper

B, L, D, H = 2, 2048, 1024, 16
HD = D // H  # 64
NCORES = 8
GROUPS = 4  # head groups per batch
GD = D // GROUPS  # 256 d' columns per group
P = 128
LT = L // P  # 16 l tiles
KD = D // P  # 8 contraction tiles for projections
NQ = L // 512  # 4 l/q chunks of 512
F32 = mybir.dt.float32
F32R = mybir.dt.float32r
BF16 = mybir.dt.bfloat16

MAX_PHASE = 4  # 2 = projections only, 3 = +attention, 4 = full (bench.py)
TRIM = True  # trim diagonal score/exp columns
DEBUG_DUMPS = False


def build_nc():
    nc = bacc.Bacc("TRN2", target_bir_lowering=False)
    xb = nc.dram_tensor("xb", [L, D], F32, kind="ExternalInput")
    wq = nc.dram_tensor("wq", [D, GD], F32, kind="ExternalInput")
    wk = nc.dram_tensor("wk", [D, GD], F32, kind="ExternalInput")
    wv = nc.dram_tensor("wv", [D, GD], F32, kind="ExternalInput")
    wo = nc.dram_tensor("wo", [GD, D], F32, kind="ExternalInput")
    y = nc.dram_tensor("y", [L, D], F32, kind="ExternalOutput")
    if DEBUG_DUMPS:
        d_QKT = nc.dram_tensor("d_QKT", [P, 2, 2, L], F32, kind="ExternalOutput")
        d_V = nc.dram_tensor("d_V", [P, LT, 4, HD + 1], BF16, kind="ExternalOutput")
        d_O = nc.dram_tensor("d_O", [P, LT, GD], F32, kind="ExternalOutput")
        d_E = nc.dram_tensor("d_E", [P, 16, 2, 512], BF16, kind="ExternalOutput")

    with tile.TileContext(nc) as tc:
        with (
            tc.tile_pool(name="const", bufs=1) as constp,
            tc.tile_pool(name="persist", bufs=1) as persist,
            tc.tile_pool(name="eallp", bufs=2) as eallp,
            tc.tile_pool(name="xTc", bufs=2) as xTcp,
            tc.tile_pool(name="otp", bufs=1) as otp,
            tc.tile_pool(name="xload", bufs=2) as xload,
            tc.tile_pool(name="ysbp", bufs=2) as ysbp,
            tc.tile_pool(name="rp", bufs=8) as rp,
            tc.tile_pool(name="psMain", bufs=3, space="PSUM") as psM,
            tc.tile_pool(name="psSmall", bufs=2, space="PSUM") as psSm,
        ):
            ident = constp.tile([P, P], F32, tag="ident")
            make_identity(nc, ident)
            # trimask[k, q] = 1 where q >= k (keep), 0 below diagonal
            trimask = constp.tile([P, P], BF16, tag="trimask")
            make_upper_triangular(nc, trimask, val=1.0, diag=True)

            wo_sb = persist.tile([P, GD // P, D], F32R, tag="wo")
            wq_sb = persist.tile([P, KD, GD], F32R, tag="wq")
            wk_sb = persist.tile([P, KD, GD], F32R, tag="wk")
            wv_sb = persist.tile([P, KD, GD], F32R, tag="wv")

            def emit_weight_dmas():
                # scalar-engine DMA queue: runs in parallel with the x loads
                # issued on the sync queue
                for t, s in ((wq_sb, wq), (wk_sb, wk), (wv_sb, wv), (wo_sb, wo)):
                    nc.scalar.dma_start(
                        t[:], s.rearrange("(ko p) n -> p ko n", p=P).bitcast(F32R)
                    )

            # QKT[:, ot, 0, :] = Q^T rows, QKT[:, ot, 1, :] = K^T rows
            QKT = persist.tile([P, 2, 2, L], F32R, tag="QKT")
            Vaug = persist.tile([P, LT, 4, HD + 1], BF16, tag="Vaug")
            nc.vector.memset(Vaug[:, :, :, HD : HD + 1], 1.0)
            O_sb = persist.tile([P, LT, GD], F32, tag="O")

            def tqk_units(j):
                """Unit list for chunk j's x^T + Q^T/K^T. PE costs in ns."""
                xTj = xTcp.tile([P, KD, 512], F32R, tag="xTc", name=f"xT{j}")
                units = []

                def lt_unit(lcl):
                    def emit():
                        lt = 4 * j + lcl
                        xt = xload.tile([P, D], F32, tag="xt", name=f"xt{lt}")
                        nc.sync.dma_start(xt[:], xb[lt * P : (lt + 1) * P, :])
                        pm = psM.tile([P, KD, P], F32, tag="m", name=f"pmt{lt}")
                        for dt_ in range(KD):
                            nc.tensor.transpose(
                                pm[:, dt_, :],
                                xt[:, dt_ * P : (dt_ + 1) * P],
                                ident[:],
                            )
                        nc.vector.tensor_copy(
                            xTj[:, :, lcl * P : (lcl + 1) * P], pm[:]
                        )

                    return emit

                def qk_unit(ot):
                    def emit():
                        pqk = psM.tile([P, 2, 512], F32, tag="m", name=f"pqk{ot}{j}")
                        for dt_ in range(KD):
                            nc.tensor.matmul(
                                pqk[:, 0, :],
                                wq_sb[:, dt_, ot * P : (ot + 1) * P],
                                xTj[:, dt_, :],
                                start=(dt_ == 0),
                                stop=(dt_ == KD - 1),
                            )
                            nc.tensor.matmul(
                                pqk[:, 1, :],
                                wk_sb[:, dt_, ot * P : (ot + 1) * P],
                                xTj[:, dt_, :],
                                start=(dt_ == 0),
                                stop=(dt_ == KD - 1),
                            )
                        nc.vector.tensor_copy(
                            QKT[:, ot, :, j * 512 : (j + 1) * 512], pqk[:]
                        )

                    return emit

                for lcl in range(4):
                    units.append((900, lt_unit(lcl)))
                if MAX_PHASE >= 2:
                    for ot in range(2):
                        units.append((3400, qk_unit(ot)))
                return xTj, units

            def v_units(j, xTj):
                if MAX_PHASE < 2:
                    return []

                def v_unit(half2):
                    def emit():
                        pv = psM.tile(
                            [P, 2, 512], F32, tag="m", name=f"pvv{j}{half2}"
                        )
                        for half in range(2):
                            lcl = 2 * half2 + half
                            for dt_ in range(KD):
                                nc.tensor.matmul(
                                    pv[:, half, 0 : 4 * HD],
                                    xTj[:, dt_, lcl * P : (lcl + 1) * P],
                                    wv_sb[:, dt_, :],
                                    start=(dt_ == 0),
                                    stop=(dt_ == KD - 1),
                                )
                        lt0 = 4 * j + 2 * half2
                        nc.vector.tensor_copy(
                            Vaug[:, lt0 : lt0 + 2, :, 0:HD],
                            pv[:, :, 0 : 4 * HD].rearrange(
                                "p a (h d) -> p a h d", h=4
                            ),
                        )

                    return emit

                return [(1800, v_unit(half2)) for half2 in range(2)]

            def alloc_e(j):
                return [
                    eallp.tile([P, 16, 2, 512], BF16, tag="eall", name=f"eall{j}{p}")
                    for p in range(2)
                ]

            def emit_score_unit(j, E_pair, p, ktg):
                """S^T matmuls + exp + causal masks for one (pair, ktg).
                Diagonal k tiles are trimmed to their valid q columns."""
                E_all = E_pair[p]
                qlo_pair = min(256, max(0, (2 * ktg - 4 * j) * P)) if TRIM else 0
                psS = [
                    psM.tile([P, 2, 512], F32, tag="m", name=f"s{j}{p}{ktg}{hh}")
                    for hh in range(2)
                ]
                for u in range(2):
                    kt = 2 * ktg + u
                    qlo = (min(256, max(0, (kt - 4 * j) * P))) if TRIM else 0
                    for h in range(2):
                        nc.tensor.matmul(
                            psS[h][:, u, qlo:512],
                            QKT[64 * h : 64 * h + 64, p, 1, kt * P : (kt + 1) * P],
                            QKT[
                                64 * h : 64 * h + 64,
                                p,
                                0,
                                j * 512 + qlo : (j + 1) * 512,
                            ],
                            start=True,
                            stop=True,
                            tile_position=(64 * h, 0),
                        )
                for h in range(2):
                    nc.scalar.activation(
                        E_all[:, 2 * ktg : 2 * ktg + 2, h, qlo_pair:512],
                        psS[h][:, :, qlo_pair:512],
                        mybir.ActivationFunctionType.Exp,
                        scale=0.125,
                    )
                    for u in range(2):
                        kt = 2 * ktg + u
                        i_diag = kt - 4 * j
                        if 0 <= i_diag < 4:
                            nc.vector.tensor_mul(
                                out=E_all[:, kt, h, i_diag * P : (i_diag + 1) * P],
                                in0=E_all[:, kt, h, i_diag * P : (i_diag + 1) * P],
                                in1=trimask[:],
                            )

            def finish_units(j, E_pair):
                """PV + normalize (per pair,head), then O^T + Wo + store
                (per l tile) for q chunk j."""
                units = []
                OTj = otp.tile([P, 2, 512], F32R, tag="otj", name=f"otj{j}")

                def pv_unit(p, h):
                    def emit():
                        E_all = E_pair[p]
                        # 4 accumulation groups share one PSUM bank. A group's
                        # start=True clears the whole bank's has_written bits,
                        # so groups must run strictly sequentially on PE —
                        # enforced with explicit ordering deps (the scheduler
                        # may otherwise reorder disjoint-subtile matmuls).
                        psO4 = psSm.tile(
                            [P, 4, HD + 1], F32, tag="o", name=f"pv{j}{p}{h}"
                        )
                        prev_last = None
                        for i in range(4):
                            for kt in range(4 * j + i + 1):
                                mm = nc.tensor.matmul(
                                    psO4[:, i, :],
                                    E_all[:, kt, h, i * P : (i + 1) * P],
                                    Vaug[:, kt, 2 * p + h, :],
                                    start=(kt == 0),
                                    stop=(kt == 4 * j + i),
                                )
                                if kt == 0 and prev_last is not None:
                                    add_dep_helper(
                                        mm.ins,
                                        prev_last.ins,
                                        sync=False,
                                        reason="pv groups share a psum bank",
                                    )
                                prev_last = mm
                        r4 = rp.tile([P, 4], F32, tag="r", name=f"r{j}{p}{h}")
                        nc.vector.reciprocal(r4[:], psO4[:, :, HD])
                        nc.vector.tensor_tensor(
                            out=O_sb[
                                :,
                                4 * j : 4 * j + 4,
                                (2 * p + h) * HD : (2 * p + h + 1) * HD,
                            ],
                            in0=psO4[:, :, 0:HD],
                            in1=r4[:, :, None].to_broadcast((P, 4, HD)),
                            op=mybir.AluOpType.mult,
                        )

                    return emit

                def out_unit(lcl):
                    def emit():
                        lt = 4 * j + lcl
                        pot = psSm.tile([P, 2, P], F32, tag="o", name=f"pot{lt}")
                        for ot in range(2):
                            nc.tensor.transpose(
                                pot[:, ot, :],
                                O_sb[:, lt, ot * P : (ot + 1) * P],
                                ident[:],
                            )
                        nc.vector.tensor_copy(
                            OTj[:, :, lcl * P : (lcl + 1) * P], pot[:]
                        )
                        ysb = ysbp.tile([P, D], F32, tag="ysb", name=f"ysb{lt}")
                        for nch in range(2):
                            psw = psSm.tile(
                                [P, 512], F32, tag="o", name=f"psw{lt}{nch}"
                            )
                            for ot in range(2):
                                nc.tensor.matmul(
                                    psw[:],
                                    OTj[:, ot, lcl * P : (lcl + 1) * P],
                                    wo_sb[:, ot, nch * 512 : (nch + 1) * 512],
                                    start=(ot == 0),
                                    stop=(ot == 1),
                                )
                            nc.vector.tensor_copy(
                                ysb[:, nch * 512 : (nch + 1) * 512], psw[:]
                            )
                        nc.sync.dma_start(y[lt * P : (lt + 1) * P, :], ysb[:])

                    return emit

                for p in range(2):
                    for h in range(2):
                        units.append((40 * (16 * j + 10), pv_unit(p, h)))
                if MAX_PHASE >= 4:
                    for lcl in range(4):
                        units.append((1750, out_unit(lcl)))
                return units

            # Greedy cost-balanced emission: per shell, interleave score
            # units (which feed ACT's exp stream) with the other PE work
            # (previous chunk's PV/output, next chunk's Q/K, this chunk's V)
            # so PE and ACT stay concurrently fed. Each shell drains both
            # queues, which also keeps the E-buffer rotation deadlock-free:
            # PV(j-1) is always fully emitted before shell j+1's exp.
            emit_weight_dmas()
            xT_cur, units0 = tqk_units(0)
            for _, emit in units0:
                emit()
            E_prev = None
            other_q = []
            oi = 0
            for j in range(NQ):
                E_cur = alloc_e(j) if MAX_PHASE >= 3 else None
                flat_scores = []
                if E_cur is not None:
                    for p in range(2):
                        for ktg in range(2 * j + 2):
                            ac = 1440.0 if ktg == 2 * j + 1 else 2300.0
                            flat_scores.append((p, ktg, ac, 850.0))
                if j + 1 < NQ:
                    xT_next, tu = tqk_units(j + 1)
                else:
                    xT_next, tu = None, []
                if E_prev is not None:
                    other_q += finish_units(j - 1, E_pair=E_prev)
                other_q += tu
                other_q += v_units(j, xT_cur)

                si = 0
                act_t, pe_t = 0.0, 0.0
                # interleave until both queues drain (per-shell barrier keeps
                # the E-buffer rotation deadlock-free: PV(j-1) is always fully
                # emitted before shell j+1's exp)
                while si < len(flat_scores) or oi < len(other_q):
                    if si < len(flat_scores) and (
                        act_t <= pe_t or oi >= len(other_q)
                    ):
                        p_, ktg_, ac, pc = flat_scores[si]
                        emit_score_unit(j, E_cur, p_, ktg_)
                        act_t += ac
                        pe_t += pc
                        si += 1
                    else:
                        cost, emit = other_q[oi]
                        emit()
                        pe_t += cost
                        oi += 1
                E_prev, xT_cur = E_cur, xT_next
            while oi < len(other_q):
                other_q[oi][1]()
                oi += 1
            if E_prev is not None:
                if DEBUG_DUMPS:
                    nc.sync.dma_start(d_E[:], E_prev[1][:])
                for _, emit in finish_units(NQ - 1, E_pair=E_prev):
                    emit()
            if DEBUG_DUMPS:
                nc.sync.dma_start(d_QKT[:], QKT[:].bitcast(F32))
                nc.sync.dma_start(d_V[:], Vaug[:])
                nc.sync.dma_start(d_O[:], O_sb[:])

    nc.compile()
    return nc


_NC_CACHE = None


def kernel(**inputs) -> np.ndarray:
    global _NC_CACHE
    x = np.asarray(inputs["x"], dtype=np.float32)
    Wq = np.asarray(inputs["Wq"], dtype=np.float32)
    Wk = np.asarray(inputs["Wk"], dtype=np.float32)
    Wv = np.asarray(inputs["Wv"], dtype=np.float32)
    Wo = np.asarray(inputs["Wo"], dtype=np.float32)

    if _NC_CACHE is None:
        _NC_CACHE = build_nc()
    nc = _NC_CACHE

    in_maps = []
    for c in range(NCORES):
        b, g = c // GROUPS, c % GROUPS
        cs = slice(g * GD, (g + 1) * GD)
        in_maps.append(
            {
                "xb": np.ascontiguousarray(x[b]),
                "wq": np.ascontiguousarray(Wq[:, cs]),
                "wk": np.ascontiguousarray(Wk[:, cs]),
                "wv": np.ascontiguousarray(Wv[:, cs]),
                "wo": np.ascontiguousarray(Wo[cs, :]),
            }
        )

    res = run_bass_kernel_spmd(nc, in_maps, core_ids=list(range(NCORES)))
    out = np.zeros((B, L, D), dtype=np.float32)
    for c in range(NCORES):
        out[c // GROUPS] += res.results[c]["y"]
    return out


# revision 40
# speedup vs baseline: 17.3504x; 1.1585x over previous
"""Multi-head causal attention (B=2, L=2048, D=1024, H=16) on 8 TRN2 cores.

Sharding: core c handles batch b = c // 4 and head group g = c % 4
(4 heads = 256 of the 1024 d' columns). Each core computes
  Q^T,K^T = (x_b @ Wq/Wk[:, g])^T, V = x_b @ Wv[:, g]
  per-head causal softmax(QK^T/8) @ V  (no max subtraction: scores ~ N(0,1))
  partial = attn_out @ Wo[g, :]
Host sums the 4 per-group partials per batch.

Structure: one fused loop over the four 512-wide l/q chunks j. Each
iteration produces that chunk's x^T / Q^T / K^T / V (PE-heavy), then runs
causal attention for both head pairs on q chunk j against k chunks <= j
(ACT-heavy exp), then the Wo output block for the finished rows — so the
PE-bound projection work of chunk j+1 overlaps the exp-bound attention of
chunk j.

Engine layout:
  PE:  transposes + all matmuls (f32r 1 cyc/row for 512-wide, PV in bf16)
  ACT: exp(S^T) from PSUM (scale=1/8), half of the projection-phase copies
  DVE: other copies, causal mask muls, reciprocal + normalize
PSUM (8 banks): "m" 3x[128,1024] slots (transposes/QKV/scores), "o"
2x[128,512] slots (PV accumulators, O^T transposes, Wo).
PSUM note: start=True clears has_written bits for the whole bank (but not
the data), so every accumulation group gets its own pool tile; only
non-accumulating outputs (transposes, paired Q/K groups in separate banks)
share a slot.
"""

import numpy as np

import concourse.bass as bass
import concourse.tile as tile
from concourse import bacc, mybir
from concourse.bass_utils import run_bass_kernel_spmd
from concourse.masks import make_identity, make_upper_triangular
from concourse.tile import add_dep_helper

B, L, D, H = 2, 2048, 1024, 16
HD = D // H  # 64
NCORES = 8
GROUPS = 4  # head groups per batch
GD = D // GROUPS  # 256 d' columns per group
P = 128
LT = L // P  # 16 l tiles
KD = D // P  # 8 contraction tiles for projections
NQ = L // 512  # 4 l/q chunks of 512
F32 = mybir.dt.float32
F32R = mybir.dt.float32r
BF16 = mybir.dt.bfloat16

MAX_PHASE = 4  # 2 = projections only, 3 = +attention, 4 = full (bench.py)
TRIM = True  # trim diagonal score/exp columns
DEBUG_DUMPS = False


def build_nc():
    nc = bacc.Bacc("TRN2", target_bir_lowering=False)
    xb = nc.dram_tensor("xb", [L, D], F32, kind="ExternalInput")
    wq = nc.dram_tensor("wq", [D, GD], F32, kind="ExternalInput")
    wk = nc.dram_tensor("wk", [D, GD], F32, kind="ExternalInput")
    wv = nc.dram_tensor("wv", [D, GD], F32, kind="ExternalInput")
    wo = nc.dram_tensor("wo", [GD, D], F32, kind="ExternalInput")
    y = nc.dram_tensor("y", [L, D], F32, kind="ExternalOutput")
    if DEBUG_DUMPS:
        d_QKT = nc.dram_tensor("d_QKT", [P, 2, 2, L], F32, kind="ExternalOutput")
        d_V = nc.dram_tensor("d_V", [P, LT, 4, HD + 1], BF16, kind="ExternalOutput")
        d_O = nc.dram_tensor("d_O", [P, LT, GD], F32, kind="ExternalOutput")
        d_E = nc.dram_tensor("d_E", [P, 16, 2, 512], BF16, kind="ExternalOutput")

    with tile.TileContext(nc) as tc:
        with (
            tc.tile_pool(name="const", bufs=1) as constp,
            tc.tile_pool(name="persist", bufs=1) as persist,
            tc.tile_pool(name="eallp", bufs=2) as eallp,
            tc.tile_pool(name="xTc", bufs=2) as xTcp,
            tc.tile_pool(name="otp", bufs=1) as otp,
            tc.tile_pool(name="xload", bufs=2) as xload,
            tc.tile_pool(name="ysbp", bufs=2) as ysbp,
            tc.tile_pool(name="rp", bufs=8) as rp,
            tc.tile_pool(name="psMain", bufs=3, space="PSUM") as psM,
            tc.tile_pool(name="psSmall", bufs=2, space="PSUM") as psSm,
        ):
            ident = constp.tile([P, P], F32, tag="ident")
            make_identity(nc, ident)
            # trimask[k, q] = 1 where q >= k (keep), 0 below diagonal
            trimask = constp.tile([P, P], BF16, tag="trimask")
            make_upper_triangular(nc, trimask, val=1.0, diag=True)

            wo_sb = persist.tile([P, GD // P, D], F32R, tag="wo")
            wq_sb = persist.tile([P, KD, GD], F32R, tag="wq")
            wk_sb = persist.tile([P, KD, GD], F32R, tag="wk")
            wv_sb = persist.tile([P, KD, GD], F32R, tag="wv")

            def emit_weight_dmas(pairs):
                # scalar-engine DMA queue; emitted after chunk 0's x loads and
                # split so only Q/K weights sit on the critical path
                for t, s in pairs:
                    nc.scalar.dma_start(
                        t[:], s.rearrange("(ko p) n -> p ko n", p=P).bitcast(F32R)
                    )

            # QKT[:, ot, 0, :] = Q^T rows, QKT[:, ot, 1, :] = K^T rows
            QKT = persist.tile([P, 2, 2, L], F32R, tag="QKT")
            Vaug = persist.tile([P, LT, 4, HD + 1], BF16, tag="Vaug")
            nc.vector.memset(Vaug[:, :, :, HD : HD + 1], 1.0)
            O_sb = persist.tile([P, LT, GD], F32, tag="O")

            def tqk_units(j):
                """Unit list for chunk j's x^T + Q^T/K^T. PE costs in ns."""
                xTj = xTcp.tile([P, KD, 512], F32R, tag="xTc", name=f"xT{j}")
                units = []

                def lt_unit(lcl):
                    def emit():
                        lt = 4 * j + lcl
                        xt = xload.tile([P, D], F32, tag="xt", name=f"xt{lt}")
                        nc.sync.dma_start(xt[:], xb[lt * P : (lt + 1) * P, :])
                        pm = psM.tile([P, KD, P], F32, tag="m", name=f"pmt{lt}")
                        for dt_ in range(KD):
                            nc.tensor.transpose(
                                pm[:, dt_, :],
                                xt[:, dt_ * P : (dt_ + 1) * P],
                                ident[:],
                            )
                        nc.vector.tensor_copy(
                            xTj[:, :, lcl * P : (lcl + 1) * P], pm[:]
                        )

                    return emit

                def qk_unit(ot):
                    def emit():
                        pqk = psM.tile([P, 2, 512], F32, tag="m", name=f"pqk{ot}{j}")
                        for dt_ in range(KD):
                            nc.tensor.matmul(
                                pqk[:, 0, :],
                                wq_sb[:, dt_, ot * P : (ot + 1) * P],
                                xTj[:, dt_, :],
                                start=(dt_ == 0),
                                stop=(dt_ == KD - 1),
                            )
                            nc.tensor.matmul(
                                pqk[:, 1, :],
                                wk_sb[:, dt_, ot * P : (ot + 1) * P],
                                xTj[:, dt_, :],
                                start=(dt_ == 0),
                                stop=(dt_ == KD - 1),
                            )
                        nc.vector.tensor_copy(
                            QKT[:, ot, :, j * 512 : (j + 1) * 512], pqk[:]
                        )

                    return emit

                for lcl in range(4):
                    units.append((900, lt_unit(lcl)))
                if MAX_PHASE >= 2:
                    for ot in range(2):
                        units.append((3400, qk_unit(ot)))
                return xTj, units

            def v_units(j, xTj):
                if MAX_PHASE < 2:
                    return []

                def v_unit(half2):
                    def emit():
                        pv = psM.tile(
                            [P, 2, 512], F32, tag="m", name=f"pvv{j}{half2}"
                        )
                        for half in range(2):
                            lcl = 2 * half2 + half
                            for dt_ in range(KD):
                                nc.tensor.matmul(
                                    pv[:, half, 0 : 4 * HD],
                                    xTj[:, dt_, lcl * P : (lcl + 1) * P],
                                    wv_sb[:, dt_, :],
                                    start=(dt_ == 0),
                                    stop=(dt_ == KD - 1),
                                )
                        lt0 = 4 * j + 2 * half2
                        nc.vector.tensor_copy(
                            Vaug[:, lt0 : lt0 + 2, :, 0:HD],
                            pv[:, :, 0 : 4 * HD].rearrange(
                                "p a (h d) -> p a h d", h=4
                            ),
                        )

                    return emit

                return [(1800, v_unit(half2)) for half2 in range(2)]

            def alloc_e(j):
                return [
                    eallp.tile([P, 16, 2, 512], BF16, tag="eall", name=f"eall{j}{p}")
                    for p in range(2)
                ]

            def emit_score_unit(j, E_pair, p, ktg):
                """S^T matmuls + exp + causal masks for one (pair, ktg).
                Diagonal k tiles are trimmed to their valid q columns."""
                E_all = E_pair[p]
                qlo_pair = min(256, max(0, (2 * ktg - 4 * j) * P)) if TRIM else 0
                psS = [
                    psM.tile([P, 2, 512], F32, tag="m", name=f"s{j}{p}{ktg}{hh}")
                    for hh in range(2)
                ]
                for u in range(2):
                    kt = 2 * ktg + u
                    qlo = (min(256, max(0, (kt - 4 * j) * P))) if TRIM else 0
                    for h in range(2):
                        nc.tensor.matmul(
                            psS[h][:, u, qlo:512],
                            QKT[64 * h : 64 * h + 64, p, 1, kt * P : (kt + 1) * P],
                            QKT[
                                64 * h : 64 * h + 64,
                                p,
                                0,
                                j * 512 + qlo : (j + 1) * 512,
                            ],
                            start=True,
                            stop=True,
                            tile_position=(64 * h, 0),
                        )
                for h in range(2):
                    nc.scalar.activation(
                        E_all[:, 2 * ktg : 2 * ktg + 2, h, qlo_pair:512],
                        psS[h][:, :, qlo_pair:512],
                        mybir.ActivationFunctionType.Exp,
                        scale=0.125,
                    )
                    for u in range(2):
                        kt = 2 * ktg + u
                        i_diag = kt - 4 * j
                        if 0 <= i_diag < 4:
                            nc.vector.tensor_mul(
                                out=E_all[:, kt, h, i_diag * P : (i_diag + 1) * P],
                                in0=E_all[:, kt, h, i_diag * P : (i_diag + 1) * P],
                                in1=trimask[:],
                            )

            def finish_units(j, E_pair):
                """PV + normalize (per pair,head), then O^T + Wo + store
                (per l tile) for q chunk j."""
                units = []
                OTj = otp.tile([P, 2, 512], F32R, tag="otj", name=f"otj{j}")

                def pv_unit(p, h):
                    def emit():
                        E_all = E_pair[p]
                        # 4 accumulation groups share one PSUM bank. A group's
                        # start=True clears the whole bank's has_written bits,
                        # so groups must run strictly sequentially on PE —
                        # enforced with explicit ordering deps (the scheduler
                        # may otherwise reorder disjoint-subtile matmuls).
                        psO4 = psSm.tile(
                            [P, 4, HD + 1], F32, tag="o", name=f"pv{j}{p}{h}"
                        )
                        prev_last = None
                        for i in range(4):
                            for kt in range(4 * j + i + 1):
                                mm = nc.tensor.matmul(
                                    psO4[:, i, :],
                                    E_all[:, kt, h, i * P : (i + 1) * P],
                                    Vaug[:, kt, 2 * p + h, :],
                                    start=(kt == 0),
                                    stop=(kt == 4 * j + i),
                                )
                                if kt == 0 and prev_last is not None:
                                    add_dep_helper(
                                        mm.ins,
                                        prev_last.ins,
                                        sync=False,
                                        reason="pv groups share a psum bank",
                                    )
                                prev_last = mm
                        r4 = rp.tile([P, 4], F32, tag="r", name=f"r{j}{p}{h}")
                        nc.vector.reciprocal(r4[:], psO4[:, :, HD])
                        nc.vector.tensor_tensor(
                            out=O_sb[
                                :,
                                4 * j : 4 * j + 4,
                                (2 * p + h) * HD : (2 * p + h + 1) * HD,
                            ],
                            in0=psO4[:, :, 0:HD],
                            in1=r4[:, :, None].to_broadcast((P, 4, HD)),
                            op=mybir.AluOpType.mult,
                        )

                    return emit

                def out_unit(lcl):
                    def emit():
                        lt = 4 * j + lcl
                        pot = psSm.tile([P, 2, P], F32, tag="o", name=f"pot{lt}")
                        for ot in range(2):
                            nc.tensor.transpose(
                                pot[:, ot, :],
                                O_sb[:, lt, ot * P : (ot + 1) * P],
                                ident[:],
                            )
                        nc.vector.tensor_copy(
                            OTj[:, :, lcl * P : (lcl + 1) * P], pot[:]
                        )
                        ysb = ysbp.tile([P, D], F32, tag="ysb", name=f"ysb{lt}")
                        for nch in range(2):
                            psw = psSm.tile(
                                [P, 512], F32, tag="o", name=f"psw{lt}{nch}"
                            )
                            for ot in range(2):
                                nc.tensor.matmul(
                                    psw[:],
                                    OTj[:, ot, lcl * P : (lcl + 1) * P],
                                    wo_sb[:, ot, nch * 512 : (nch + 1) * 512],
                                    start=(ot == 0),
                                    stop=(ot == 1),
                                )
                            nc.vector.tensor_copy(
                                ysb[:, nch * 512 : (nch + 1) * 512], psw[:]
                            )
                        nc.sync.dma_start(y[lt * P : (lt + 1) * P, :], ysb[:])

                    return emit

                for p in range(2):
                    for h in range(2):
                        units.append((40 * (16 * j + 10), pv_unit(p, h)))
                if MAX_PHASE >= 4:
                    for lcl in range(4):
                        units.append((1750, out_unit(lcl)))
                return units

            # Greedy cost-balanced emission: per shell, interleave score
            # units (which feed ACT's exp stream) with the other PE work
            # (previous chunk's PV/output, next chunk's Q/K, this chunk's V).
            # Each shell drains both queues, which keeps the E-buffer
            # rotation deadlock-free (PV(j-1) always fully emitted before
            # shell j+1's exp).
            xT_cur, units0 = tqk_units(0)
            for _, emit in units0[:4]:  # x loads + transposes first
                emit()
            emit_weight_dmas([(wq_sb, wq), (wk_sb, wk)])
            for _, emit in units0[4:]:
                emit()
            emit_weight_dmas([(wv_sb, wv), (wo_sb, wo)])

            E_prev = None
            other_q = []
            oi = 0
            tail_units = []
            for j in range(NQ):
                E_cur = alloc_e(j) if MAX_PHASE >= 3 else None
                flat_scores = []
                if E_cur is not None:
                    for p in range(2):
                        for ktg in range(2 * j + 2):
                            ac = 1440.0 if ktg == 2 * j + 1 else 2300.0
                            flat_scores.append((p, ktg, ac, 850.0))
                if j + 1 < NQ:
                    xT_next, tu = tqk_units(j + 1)
                else:
                    xT_next, tu = None, []
                if E_prev is not None:
                    other_q += finish_units(j - 1, E_pair=E_prev)
                other_q += tu
                other_q += v_units(j, xT_cur)

                si = 0
                act_t, pe_t = 0.0, 0.0
                while si < len(flat_scores) or oi < len(other_q):
                    if si < len(flat_scores) and (
                        act_t <= pe_t or oi >= len(other_q)
                    ):
                        p_, ktg_, ac, pc = flat_scores[si]
                        emit_score_unit(j, E_cur, p_, ktg_)
                        act_t += ac
                        pe_t += pc
                        si += 1
                    else:
                        cost, emit = other_q[oi]
                        emit()
                        pe_t += cost
                        oi += 1
                E_prev, xT_cur = E_cur, xT_next
            if E_prev is not None:
                if DEBUG_DUMPS:
                    nc.sync.dma_start(d_E[:], E_prev[1][:])
                for _, emit in finish_units(NQ - 1, E_pair=E_prev):
                    emit()
            if DEBUG_DUMPS:
                nc.sync.dma_start(d_QKT[:], QKT[:].bitcast(F32))
                nc.sync.dma_start(d_V[:], Vaug[:])
                nc.sync.dma_start(d_O[:], O_sb[:])

    nc.compile()
    return nc


_NC_CACHE = None


def kernel(**inputs) -> np.ndarray:
    global _NC_CACHE
    x = np.asarray(inputs["x"], dtype=np.float32)
    Wq = np.asarray(inputs["Wq"], dtype=np.float32)
    Wk = np.asarray(inputs["Wk"], dtype=np.float32)
    Wv = np.asarray(inputs["Wv"], dtype=np.float32)
    Wo = np.asarray(inputs["Wo"], dtype=np.float32)

    if _NC_CACHE is None:
        _NC_CACHE = build_nc()
    nc = _NC_CACHE

    in_maps = []
    for c in range(NCORES):
        b, g = c // GROUPS, c % GROUPS
        cs = slice(g * GD, (g + 1) * GD)
        in_maps.append(
            {
                "xb": np.ascontiguousarray(x[b]),
                "wq": np.ascontiguousarray(Wq[:, cs]),
                "wk": np.ascontiguousarray(Wk[:, cs]),
                "wv": np.ascontiguousarray(Wv[:, cs]),
                "wo": np.ascontiguousarray(Wo[cs, :]),
            }
        )

    res = run_bass_kernel_spmd(nc, in_maps, core_ids=list(range(NCORES)))
    out = np.zeros((B, L, D), dtype=np.float32)
    for c in range(NCORES):
        out[c // GROUPS] += res.results[c]["y"]
    return out
